# revision 58
# baseline (speedup 1.0000x reference)
"""Trainium2 Bass kernel for nn_Block_15650860827274 (dense transformer block).

Sharding: DP-8 over (batch b, query-half j). Core c = 2*b + j handles batch b
and query positions [256*j, 256*j+256). The sequence axis is rotated on the
host so every core's own queries are columns 0:256 of its (transposed) input;
K/V are computed for the full (permuted) sequence on-device.

Speed levers over the f16 baseline:
- fp8e4 DoubleRow matmuls (0.5 cyc/col, 256-row contraction) for Q/K/V/O
  projections and both FFN layers. FFN precision is recovered with
  same-scale residual compensation: FFN1 runs 3 passes (h_hi@W1hi +
  h_lo@W1hi + (h_hi/64)@W1lo_x64), FFN2 runs 2 passes (g@W2hi + g@W2lo).
- attention bias+mask enter PSUM via fp8 ident-DoubleRow matmuls against
  zero-interleaved buffers (no Pool combine pass, half the moving bytes).
- QK^T stays f16 (64-deep contraction can't pair for DoubleRow).
- probs/V/ctx in fp8; softmax normalization rides the v ones-row.
- DMA split across the SP / Activation / Pool queues (each modeled at
  ~360 GB/s); weights are host-packed fp8 pair-interleaved, fully
  contiguous per partition.
- src shipped f16; AdaLN table (silu(sinemb) @ W_ada + b_ada) folded on
  the host; stats matmuls use f16/f32r moving operands.
- dummy matmuls at t=0 ramp the PE p-state before real work arrives.
"""

import math
import sys

import numpy as np

sys.path.insert(0, "/opt/trn_rl_repo")

import ml_dtypes  # noqa: E402

import concourse.bass as bass  # noqa: E402
import concourse.bacc as bacc  # noqa: E402
import concourse.mybir as mybir  # noqa: E402
from concourse.tile import TileContext  # noqa: E402

F32 = mybir.dt.float32
F32R = mybir.dt.float32r
F16 = mybir.dt.float16
F8 = mybir.dt.float8e4
U8 = mybir.dt.uint8
I32 = mybir.dt.int32
AF = mybir.ActivationFunctionType
OP = mybir.AluOpType
DR = mybir.MatmulPerfMode.DoubleRow
E4 = ml_dtypes.float8_e4m3

B, S, D, H, HD, F = 4, 512, 1024, 16, 64, 4096
SQ = S // 2          # query positions per core
NC = 8               # cores
DC = D // 128        # 8 feature chunks
FC = F // 128        # 32 hidden chunks
KB = S // 128        # 4 key blocks
EPS = 1e-5
NUM_STEPS = 100

# fp8 scales (all powers of two; products must stay under e4m3 max 240)
SX = 8.0             # xT8 = SX * xT
SW = 64.0            # Wq/Wk/Wv/Wo
SV = 32.0            # v8 = SV * (v + bv)
SC = 32.0            # ctx8 = SC * ctx   (via ones-row u = SV/SC = 1)
SH = 8.0             # h_hi8 = SH * h
SW1 = 4.0            # W1hi
SW1L = 256.0         # W1lo stored at SW1*64; moving h_hi/64 compensates
SG = SH * SW1        # gT8 = SG * gelu2(y+b1)  (= 32)
SW2 = 64.0           # W2hi and W2lo (same-scale residual)
MASKV = -160.0       # mask stored as -160*mask at "scale 8" -> -20 in logits
IDENTV = 0.125       # ident-DR stationary value: 0.125 * 8*bias = bias
EXP_SHIFT = math.log(256.0) - 8.0   # probs8 = 256*exp(l-8); factor cancels


def _silu_table():
    half = D // 2
    freqs = np.exp(
        np.arange(half, dtype=np.float32) * np.float32(-math.log(10000.0) / (half - 1))
    ).astype(np.float32)
    t = np.arange(NUM_STEPS, dtype=np.float32)
    x = (t / np.float32(NUM_STEPS) * np.float32(4000.0)).astype(np.float32)
    e = (x[:, None] * freqs[None, :]).astype(np.float32).astype(np.float64)
    emb = np.concatenate([np.sin(e), np.cos(e)], axis=-1)
    silu = emb / (1.0 + np.exp(-emb))
    return silu.astype(np.float32)  # [100, 1024]


def _pm(vec, cols, mul=1.0):
    v = np.asarray(vec, dtype=np.float32) * np.float32(mul)
    return np.ascontiguousarray(v.reshape(cols, 128).T)


def _pack_dr(w, scale):
    """[D_in, N] f32 -> [D_in//256, 128, 2, N] fp8 pair-interleaved."""
    w = np.asarray(w, dtype=np.float32) * np.float32(scale)
    din, n = w.shape
    return np.ascontiguousarray(
        w.reshape(din // 256, 2, 128, n).transpose(0, 2, 1, 3)
    ).astype(E4)


def _pack_dr_res(w, s_hi, s_lo_mul):
    """hi at s_hi, lo = (s_hi*w - hi) at s_hi*s_lo_mul, both DR-packed fp8."""
    w = np.asarray(w, dtype=np.float32)
    hi = (w * np.float32(s_hi)).astype(E4)
    lo = ((w * np.float32(s_hi) - hi.astype(np.float32)) * np.float32(s_lo_mul)).astype(E4)

    def pk(a):
        din, n = a.shape
        return np.ascontiguousarray(
            a.reshape(din // 256, 2, 128, n).transpose(0, 2, 1, 3)
        )

    return pk(hi), pk(lo)


_NC_CACHE = {}


def build_nc():
    if "nc" in _NC_CACHE:
        return _NC_CACHE["nc"]
    nc = bacc.Bacc(
        "TRN2", target_bir_lowering=False, debug=False, num_devices=NC
    )

    # ---- I/O ----
    srcT_d = nc.dram_tensor("srcT", [DC, 128, S], F16, kind="ExternalInput")
    # bias: per head-pair hp: [KB, 128, 2*SQ] with the two heads' q-rows
    # interleaved so DMA elems are 512B
    biasT_d = nc.dram_tensor("biasT", [H // 2, KB, 128, 2 * SQ], F8, kind="ExternalInput")
    maskT_d = nc.dram_tensor("maskT", [KB, 128, SQ], F8, kind="ExternalInput")
    tstep_d = nc.dram_tensor("tstep", [1, 1], I32, kind="ExternalInput")
    tbl_d = nc.dram_tensor("tbl", [NUM_STEPS, 2 * D], F16, kind="ExternalInput")
    iota_d = nc.dram_tensor("iota100", [NUM_STEPS, 1], I32, kind="ExternalInput")
    id8_d = nc.dram_tensor("id8", [128, 2, 128], F8, kind="ExternalInput")
    wq_d = nc.dram_tensor("Wq8", [DC // 2, 128, 2, D], F8, kind="ExternalInput")
    wk_d = nc.dram_tensor("Wk8", [DC // 2, 128, 2, D], F8, kind="ExternalInput")
    wv_d = nc.dram_tensor("Wv8", [DC // 2, 128, 2, D], F8, kind="ExternalInput")
    wo_d = nc.dram_tensor("Wo8", [DC // 2, 128, 2, D], F8, kind="ExternalInput")
    # W1: [quarter, hi/lo, cp, 128, 2, F//4]
    w1_d = nc.dram_tensor("W18", [4, 2, DC // 2, 128, 2, F // 4], F8, kind="ExternalInput")
    # W2: [group, cp-in-group, 128, 2, D] split into hi and lo tensors
    w2hi_d = nc.dram_tensor("W2hi8", [4, 4, 128, 2, D], F8, kind="ExternalInput")
    w2lo_d = nc.dram_tensor("W2lo8", [4, 4, 128, 2, D], F8, kind="ExternalInput")
    # packed per-partition consts [128, 128] f32:
    # cols 0:8 bq/8 | 8:16 bk | 16:24 -8*bo | 24:32 bo | 32:40 b2 | 40:48 8*g2
    # | 48:56 8*beta2 | 56:64 unused | 64:96 1.702*b1 | 96:128 32*b1
    consts_d = nc.dram_tensor("consts_pm", [128, 128], F32, kind="ExternalInput")
    bada_d = nc.dram_tensor("bada_pm", [128, 16], F32, kind="ExternalInput")
    brow_d = nc.dram_tensor("brow", [1, 2 * D], F16, kind="ExternalInput")
    bv_d = nc.dram_tensor("bv32_row", [1, D], F16, kind="ExternalInput")
    out_d = nc.dram_tensor("outT", [DC, 128, SQ], F32, kind="ExternalOutput")

    with TileContext(nc) as tc:
        with (
            nc.allow_low_precision(reason="fp8/f16 paths are error-analyzed"),
            tc.tile_pool(name="consts", bufs=1) as cpool,
            tc.tile_pool(name="acts", bufs=1) as acts,
            tc.tile_pool(name="wqkvo", bufs=3) as wqkvo,
            tc.tile_pool(name="w1p", bufs=3) as w1p,
            tc.tile_pool(name="w2hip", bufs=2) as w2hip,
            tc.tile_pool(name="w2lop", bufs=2) as w2lop,
            tc.tile_pool(name="probs", bufs=2) as probsp,
            tc.tile_pool(name="smalls", bufs=3) as smalls,
            tc.tile_pool(name="st", bufs=3) as stp,
            tc.tile_pool(name="stb", bufs=2) as stbp,
            tc.tile_pool(name="ovl1", bufs=1) as ovl1,
            tc.tile_pool(name="ovl2", bufs=1) as ovl2,
            tc.tile_pool(name="pstat", bufs=2, space="PSUM") as pstat,
            tc.tile_pool(name="pbig", bufs=2, space="PSUM") as pbig,
            tc.tile_pool(name="psc", bufs=2, space="PSUM") as psc,
        ):
            # ---------------- consts + PE warmup ----------------
            ones_h = cpool.tile([128, 1], F16, tag="onesh")
            nc.vector.memset(ones_h[:], 1.0)
            cshift = cpool.tile([128, 1], F32, tag="cshift")
            nc.vector.memset(cshift[:], EXP_SHIFT)
            epsc = cpool.tile([1, 1], F32, tag="epsc")
            nc.vector.memset(epsc[:], EPS)
            warm0 = cpool.tile([1, 4], F32, tag="warm0")
            nc.scalar.activation(warm0[:, 0:1], epsc[:], AF.Sqrt)
            junk = cpool.tile([128, 512], F16, tag="junk")
            nc.vector.memset(junk[:], 0.001)
            zeros16 = cpool.tile([128, 128], F16, tag="zeros16")
            nc.vector.memset(zeros16[:], 0.0)
            warm_ps = psc.tile([128, 512], F32, tag="psc", name="warmps")
            for i in range(6):
                nc.tensor.matmul(
                    warm_ps[:], junk[:, 0:128], junk[:],
                    start=(i == 0), stop=(i == 5),
                )

            # ---------------- SP queue: src first, then weights ----------------
            srcT = acts.tile([128, DC, S], F16, tag="srcT")
            for hh in range(2):
                nc.sync.dma_start(
                    out=srcT[:, 4 * hh : 4 * (hh + 1), :],
                    in_=srcT_d[4 * hh : 4 * (hh + 1)].rearrange("c p s -> p c s"),
                )
            wq_t = wqkvo.tile([128, DC // 2, 2, D], F8, name="wqkvot")
            nc.sync.dma_start(out=wq_t[:], in_=wq_d[:].rearrange("c p a n -> p c a n"))
            wk_t = wqkvo.tile([128, DC // 2, 2, D], F8, name="wqkvot")
            nc.sync.dma_start(out=wk_t[:], in_=wk_d[:].rearrange("c p a n -> p c a n"))
            wv_t = wqkvo.tile([128, DC // 2, 2, D], F8, name="wqkvot")
            nc.sync.dma_start(out=wv_t[:], in_=wv_d[:].rearrange("c p a n -> p c a n"))
            wo_t = wqkvo.tile([128, DC // 2, 2, D], F8, name="wqkvot")
            nc.sync.dma_start(out=wo_t[:], in_=wo_d[:].rearrange("c p a n -> p c a n"))

            # ---------------- ACT queue: small loads ----------------
            t_sb = cpool.tile([1, 1], I32, tag="tsb")
            nc.scalar.dma_start(out=t_sb[:], in_=tstep_d[:])
            iota_pm = cpool.tile([NUM_STEPS, 1], I32, tag="iota")
            nc.scalar.dma_start(out=iota_pm[:], in_=iota_d[:])
            tbl_sb = cpool.tile([NUM_STEPS, 2 * D], F16, tag="tbl")
            nc.scalar.dma_start(out=tbl_sb[:], in_=tbl_d[:])
            consts_sb = cpool.tile([128, 128], F32, tag="consts")
            nc.scalar.dma_start(out=consts_sb[:], in_=consts_d[:])
            bada_sb = cpool.tile([128, 16], F32, tag="bada")
            nc.scalar.dma_start(out=bada_sb[:], in_=bada_d[:])
            id8 = cpool.tile([128, 2, 128], F8, tag="id8")
            nc.scalar.dma_start(out=id8[:], in_=id8_d[:])
            bv_row = stp.tile([1, D], F16, tag="st", name="bvrow")
            nc.scalar.dma_start(out=bv_row[:], in_=bv_d[:])
            brow = cpool.tile([1, 2 * D], F16, tag="brow")
            nc.scalar.dma_start(out=brow[:], in_=brow_d[:])
            warm = cpool.tile([1, 4], F32, tag="warm")
            ones_row = cpool.tile([1, SQ], F16, tag="onesrow")
            nc.vector.memset(ones_row[:], 1.0)
            c64 = cpool.tile([128, SQ], F16, tag="c64")
            nc.vector.memset(c64[:], 1.0 / 64.0)


            bq8_c = consts_sb[:, 0:8]
            bk_c = consts_sb[:, 8:16]
            bon8_c = consts_sb[:, 16:24]
            bo_c = consts_sb[:, 24:32]
            b2_c = consts_sb[:, 32:40]
            g28_c = consts_sb[:, 40:48]
            beta8_c = consts_sb[:, 48:56]
            b1sig_c = consts_sb[:, 64:96]
            b132_c = consts_sb[:, 96:128]
            bk512_c = consts_sb[:, 56:64]
            recip512 = cpool.tile([128, S], F8, tag="recip512")
            nc.vector.memset(recip512[:], 1.0 / (SX * SW))

            # ---------------- Pool queue: mask, bias pairs (W2lo later) -------
            maskz = cpool.tile([128, 2, KB, SQ], F8, tag="maskz")
            nc.vector.memset(maskz[:, 1, :, :], 0.0)
            nc.gpsimd.dma_start(
                out=maskz[:, 0, :, :], in_=maskT_d[:].rearrange("a p q -> p a q")
            )
            # combined bias buffer, planes [dataA, zeros, dataB]: the DR ident
            # trick reads (data, zero) or (zero, data) pairs; id8 has the
            # ident in both pair slots so order doesn't matter.
            bias3 = cpool.tile([128, 3, 2, KB, 2, SQ], F8, tag="bias3")
            nc.gpsimd.memset(bias3[:, 1, 0], 0.0)
            nc.gpsimd.memset(bias3[:, 1, 1], 0.0)

            t_b = cpool.tile([NUM_STEPS, 1], I32, tag="tb")
            nc.gpsimd.partition_broadcast(t_b[:], t_sb[:])
            onehot = cpool.tile([NUM_STEPS, 1], F16, tag="onehot")
            nc.vector.tensor_tensor(
                out=onehot[:], in0=iota_pm[:], in1=t_b[:], op=OP.is_equal
            )

            def bias_dma(g):
                nc.gpsimd.dma_start(
                    out=bias3[:, 2 * (g % 2)].rearrange("p h a b q -> p (h a b q)"),
                    in_=biasT_d[2 * g : 2 * g + 2].rearrange(
                        "h a p q -> p (h a) q"
                    ),
                )

            bias_dma(0)

            src2 = acts.tile([128, DC, S], F16, tag="kT", name="src2")
            for c in range(DC):
                nc.vector.tensor_mul(src2[:, c, :], srcT[:, c, :], srcT[:, c, :])
            sum_x = pstat.tile([1, S], F32, tag="pstat")
            for c in range(DC):
                nc.tensor.matmul(
                    sum_x[:], ones_h[:], srcT[:, c, :],
                    start=(c == 0), stop=(c == DC - 1),
                )
            sum_x2 = pstat.tile([1, S], F32, tag="pstat")
            for c in range(DC):
                nc.tensor.matmul(
                    sum_x2[:], ones_h[:], src2[:, c, :],
                    start=(c == 0), stop=(c == DC - 1),
                )
            # ---------------- timestep embedding (host-folded table) ---------
            emb_ps = psc.tile([128, 16], F32, tag="psc", name="embps")
            for i in range(16):
                nc.tensor.matmul(
                    emb_ps[:, i : i + 1],
                    tbl_sb[:, 128 * i : 128 * (i + 1)],
                    onehot[:],
                    start=True, stop=True,
                )
            ss_pm = cpool.tile([128, 16], F32, tag="sspm")
            nc.vector.tensor_add(ss_pm[:], emb_ps[:], bada_sb[:])
            scale1p = cpool.tile([128, DC], F32, tag="scale1p")
            nc.vector.tensor_scalar_add(scale1p[:], ss_pm[:, 0:DC], 1.0)
            scale1p8 = cpool.tile([128, DC], F32, tag="scale1p8")
            nc.vector.tensor_scalar_mul(scale1p8[:], scale1p[:], SX)
            shift8 = cpool.tile([128, DC], F32, tag="shift8")
            nc.vector.tensor_scalar_mul(shift8[:], ss_pm[:, DC:16], SX)

            # ---------------- LN1 stats ----------------
            mean1h = stp.tile([1, S], F16, tag="st", name="mean1h")
            nc.vector.tensor_scalar_mul(mean1h[:], sum_x[:], 1.0 / D)
            var1 = stp.tile([1, S], F32, tag="st")
            nc.vector.tensor_mul(var1[:], mean1h[:], mean1h[:])
            nc.vector.scalar_tensor_tensor(
                out=var1[:], in0=sum_x2[:], scalar=1.0 / D, in1=var1[:],
                op0=OP.mult, op1=OP.subtract,
            )
            sd1 = stp.tile([1, S], F32, tag="st", name="sd1")
            nc.scalar.activation(sd1[:], var1[:], AF.Sqrt, bias=epsc[:])
            rstd1 = stp.tile([1, S], F16, tag="st", name="rstd1")
            nc.vector.reciprocal(rstd1[:], sd1[:])
            mean1_b = stbp.tile([128, S], F16, tag="stb")
            nc.gpsimd.partition_broadcast(mean1_b[:], mean1h[:])
            rstd1_b = stbp.tile([128, S], F16, tag="stb")
            nc.gpsimd.partition_broadcast(rstd1_b[:], rstd1[:])

            # PE keepalive while DVE builds xT
            ka_ps = psc.tile([128, 512], F32, tag="psc", name="kaps")
            for i in range(16):
                nc.tensor.matmul(
                    ka_ps[:], junk[:, 0:128], junk[:],
                    start=(i == 0), stop=(i == 15),
                )

            # ---------------- xT ----------------
            # half0 (queries): xT f32 (with +bo fold = xTbo) and xT8
            # half1 (other keys): xT8 only
            xTbo = acts.tile([128, DC, SQ], F16, tag="xTbo")
            xT8 = acts.tile([128, DC, S], F8, tag="xT8")
            sl0 = slice(0, SQ)
            sl1 = slice(SQ, S)
            t0s = {}
            for c in range(DC):
                xm = smalls.tile([128, SQ], F16, tag="xm", bufs=4, name="xm0")
                nc.gpsimd.tensor_sub(xm[:], srcT[:, c, sl0], mean1_b[:, sl0])
                t0 = smalls.tile([128, SQ], F16, tag="t0", bufs=4, name="t0")
                nc.vector.scalar_tensor_tensor(
                    out=t0[:], in0=xm[:],
                    scalar=scale1p[:, c : c + 1], in1=rstd1_b[:, sl0],
                    op0=OP.mult, op1=OP.mult,
                )
                nc.scalar.activation(
                    xT8[:, c, sl0], t0[:], AF.Identity,
                    bias=shift8[:, c : c + 1], scale=SX,
                )
                t0s[c] = t0
                if c >= 3:
                    cc = c - 3
                    nc.vector.tensor_scalar_add(
                        xTbo[:, cc, :], t0s.pop(cc)[:],
                        ss_pm[:, DC + cc : DC + cc + 1],
                    )
            for cc in sorted(t0s):
                nc.vector.tensor_scalar_add(
                    xTbo[:, cc, :], t0s[cc][:], ss_pm[:, DC + cc : DC + cc + 1]
                )
            for c in range(DC):
                xm = smalls.tile([128, SQ], F16, tag="xm", bufs=4, name="xm1")
                nc.gpsimd.tensor_sub(xm[:], srcT[:, c, sl1], mean1_b[:, sl1])
                t8 = smalls.tile([128, SQ], F32, tag="t8", bufs=4, name="t8")
                nc.vector.scalar_tensor_tensor(
                    out=t8[:], in0=xm[:],
                    scalar=scale1p8[:, c : c + 1], in1=rstd1_b[:, sl1],
                    op0=OP.mult, op1=OP.mult,
                )
                nc.vector.tensor_scalar_add(
                    xT8[:, c, sl1], t8[:], shift8[:, c : c + 1]
                )

            # ---------------- Q, K projections (DR) ----------------
            qT = acts.tile([128, DC, SQ], F16, tag="qT")
            for m in range(DC):
                ps = pbig.tile([128, 4, SQ], F32, tag="pbig", name="psq")
                pq = ps[:, 0, :]
                for cp in range(DC // 2):
                    nc.tensor.matmul(
                        pq,
                        wq_t[:, cp, :, 128 * m : 128 * (m + 1)],
                        xT8[:, 2 * cp : 2 * cp + 2, 0:SQ],
                        start=(cp == 0), stop=(cp == DC // 2 - 1),
                        perf_mode=DR,
                    )
                nc.scalar.activation(
                    qT[:, m, :], pq, AF.Identity,
                    bias=bq8_c[:, m : m + 1], scale=1.0 / (SX * SW * 8.0),
                )
            kT = acts.tile([128, DC, S], F16, tag="kT")
            for m in range(DC):
                ps = pbig.tile([128, 4, SQ], F32, tag="pbig", name="psk")
                pk = ps[:].rearrange("p a q -> p (a q)")[:, 0:S]
                for cp in range(DC // 2):
                    nc.tensor.matmul(
                        pk,
                        wk_t[:, cp, :, 128 * m : 128 * (m + 1)],
                        xT8[:, 2 * cp : 2 * cp + 2, :],
                        start=(cp == 0), stop=(cp == DC // 2 - 1),
                        perf_mode=DR,
                    )
                if m % 2 == 0:
                    nc.scalar.activation(
                        kT[:, m, :], pk, AF.Identity,
                        bias=bk_c[:, m : m + 1], scale=1.0 / (SX * SW),
                    )
                else:
                    nc.vector.scalar_tensor_tensor(
                        out=kT[:, m, :], in0=pk,
                        scalar=bk512_c[:, m : m + 1], in1=recip512[:],
                        op0=OP.add, op1=OP.mult,
                    )

            bv32_b = cpool.tile([128, D], F16, tag="bvb")
            nc.gpsimd.partition_broadcast(bv32_b[:], bv_row[:])
            w2lo_tiles = [None] * 4
            for g in range(2):
                wlo = w2lop.tile([128, 4, 2, D], F8, name="w2lot")
                nc.sync.dma_start(
                    out=wlo[:], in_=w2lo_d[g].rearrange("c p a n -> p c a n")
                )
                w2lo_tiles[g] = wlo

            # ---------------- V projection (token-major, DR) ----------------
            v_sb = acts.tile([128, KB, H, HD + 1], F8, tag="v")
            nc.vector.memset(v_sb[:, :, :, HD : HD + 1], 1.0)  # SV/SC = 1
            for t in range(KB):
                for half in range(2):
                    ps = pbig.tile([128, 4, SQ], F32, tag="pbig", name="psv")
                    pv_ = ps[:].rearrange("p a q -> p (a q)")[:, 0:512]
                    for cp in range(DC // 2):
                        nc.tensor.matmul(
                            pv_,
                            xT8[:, 2 * cp : 2 * cp + 2, 128 * t : 128 * (t + 1)],
                            wv_t[:, cp, :, 512 * half : 512 * (half + 1)],
                            start=(cp == 0), stop=(cp == DC // 2 - 1),
                            perf_mode=DR,
                        )
                    ev = nc.vector
                    ev.scalar_tensor_tensor(
                        out=v_sb[:, t, 8 * half : 8 * (half + 1), 0:HD],
                        in0=pv_.rearrange("p (h d) -> p h d", h=8),
                        scalar=SV / (SX * SW),
                        in1=bv32_b[:, 512 * half : 512 * (half + 1)].rearrange(
                            "p (h d) -> p h d", h=8
                        ),
                        op0=OP.mult, op1=OP.add,
                    )

            # ---------------- attention, per head ----------------
            ctx8 = acts.tile([128, DC, SQ], F8, tag="ctx8")
            for h in range(H):
                hc, hr = h // 2, 64 * (h % 2)
                if h == 0:
                    bias_dma(1)
                if h in (4, 8):
                    bias_dma(h // 4 + 1)
                z = (h // 4) % 2  # buffer A -> planes 0:2, buffer B -> 1:3
                bias_mv = bias3[:, z : z + 2, (h // 2) % 2, :, h % 2, :]
                probs = probsp.tile([128, KB, SQ], F8, tag="probs")
                sc = pbig.tile([128, KB, SQ], F32, tag="pbig", name="scps")
                for kc in range(KB):
                    nc.tensor.matmul(
                        sc[:, kc, :],
                        id8[:],
                        bias_mv[:, :, kc, :],
                        start=True, stop=False, perf_mode=DR,
                    )
                    nc.tensor.matmul(
                        sc[:, kc, :],
                        id8[:],
                        maskz[:, :, kc, :],
                        start=False, stop=False, perf_mode=DR,
                    )
                    nc.tensor.matmul(
                        sc[:, kc, :],
                        kT[hr : hr + 64, hc, 128 * kc : 128 * (kc + 1)],
                        qT[hr : hr + 64, hc, :],
                        start=False, stop=True,
                    )
                nc.scalar.activation(
                    probs[:].rearrange("p a q -> p (a q)"),
                    sc[:].rearrange("p a q -> p (a q)"),
                    AF.Exp, bias=cshift[:],
                )
                cps = psc.tile([128, SQ], F32, tag="psc", name="cps")[: HD + 1]
                for jp in range(KB // 2):
                    nc.tensor.matmul(
                        cps,
                        v_sb[:, 2 * jp : 2 * jp + 2, h, :],
                        probs[:, 2 * jp : 2 * jp + 2, :],
                        start=(jp == 0), stop=(jp == KB // 2 - 1),
                        perf_mode=DR,
                    )
                rh = smalls.tile([1, SQ], F32, tag="rh", bufs=2)
                nc.vector.reciprocal(rh[:], cps[HD : HD + 1, :])
                last_rh = rh
                rh_b = smalls.tile([64, SQ], F32, tag="rhb", bufs=2)
                nc.gpsimd.partition_broadcast(rh_b[:], rh[:])
                nc.vector.tensor_mul(
                    ctx8[hr : hr + 64, hc, :], cps[0:HD, :], rh_b[:]
                )

            nc.scalar.activation(warm[:, 1:2], last_rh[0:1, 0:1], AF.Sqrt)

            # SP queue: W1 quarters (2MB each: hi+lo)
            w1_tiles = []
            for q in range(4):
                wt = w1p.tile([128, 2, DC // 2, 2, F // 4], F8, name="w1t")
                nc.sync.dma_start(
                    out=wt[:], in_=w1_d[q].rearrange("l c p a n -> p l c a n")
                )
                w1_tiles.append(wt)

            # ---------------- out projection + residual ----------------
            x_after = acts.tile([128, DC, SQ], F16, tag="xaf")
            xsq = acts.tile([128, DC, SQ], F16, tag="srcT", name="xsq")
            for m in range(DC):
                ps = pbig.tile([128, 4, SQ], F32, tag="pbig", name="pso")
                po = ps[:, 0, :]
                for cp in range(DC // 2):
                    nc.tensor.matmul(
                        po,
                        wo_t[:, cp, :, 128 * m : 128 * (m + 1)],
                        ctx8[:, 2 * cp : 2 * cp + 2, :],
                        start=(cp == 0), stop=False,
                        perf_mode=DR,
                    )
                nc.tensor.matmul(
                    po, brow[:, 128 * m : 128 * (m + 1)], ones_row[:],
                    start=False, stop=True,
                )
                nc.vector.scalar_tensor_tensor(
                    out=x_after[:, m, :], in0=po, scalar=1.0 / (SC * SW),
                    in1=xTbo[:, m, :], op0=OP.mult, op1=OP.add,
                )
                nc.gpsimd.tensor_mul(
                    xsq[:, m, :], x_after[:, m, :], x_after[:, m, :]
                )

            # ---------------- LN2 + h quantization ----------------
            sum2_x = pstat.tile([1, S], F32, tag="pstat", name="s2x")[:, :SQ]
            for c in range(DC):
                nc.tensor.matmul(
                    sum2_x, ones_h[:], x_after[:, c, :],
                    start=(c == 0), stop=(c == DC - 1),
                )
            sum2_x2 = pstat.tile([1, S], F32, tag="pstat", name="s2x2")[:, :SQ]
            for c in range(DC):
                nc.tensor.matmul(
                    sum2_x2, ones_h[:], xsq[:, c, :],
                    start=(c == 0), stop=(c == DC - 1),
                )
            mean2h = stp.tile([1, SQ], F16, tag="st", name="mean2h")
            nc.vector.tensor_scalar_mul(mean2h[:], sum2_x, 1.0 / D)
            var2 = stp.tile([1, SQ], F32, tag="st")
            nc.vector.tensor_mul(var2[:], mean2h[:], mean2h[:])
            nc.vector.scalar_tensor_tensor(
                out=var2[:], in0=sum2_x2, scalar=1.0 / D, in1=var2[:],
                op0=OP.mult, op1=OP.subtract,
            )
            sd2 = stp.tile([1, SQ], F32, tag="st", name="sd2")
            nc.scalar.activation(sd2[:], var2[:], AF.Sqrt, bias=epsc[:])
            rstd2 = stp.tile([1, SQ], F16, tag="st", name="rstd2")
            nc.vector.reciprocal(rstd2[:], sd2[:])
            nc.scalar.activation(warm[:, 2:3], rstd2[0:1, 0:1], AF.Sigmoid)
            mean2_b = stbp.tile([128, SQ], F16, tag="stb2", bufs=1)
            nc.gpsimd.partition_broadcast(mean2_b[:], mean2h[:])
            rstd2_b = stbp.tile([128, SQ], F16, tag="stb2b", bufs=1)
            nc.gpsimd.partition_broadcast(rstd2_b[:], rstd2[:])

            h_hi = acts.tile([128, DC, SQ], F8, tag="hhi")
            h_hi64 = acts.tile([128, DC, SQ], F8, tag="hhi64")
            h_lo = acts.tile([128, DC, SQ], F8, tag="hlo")
            for c in range(DC):
                xm2 = smalls.tile([128, SQ], F16, tag="xm", bufs=4, name="xm2")
                nc.gpsimd.tensor_sub(xm2[:], x_after[:, c, :], mean2_b[:])
                t2 = smalls.tile([128, SQ], F32, tag="t8", bufs=4, name="t2")
                nc.vector.scalar_tensor_tensor(
                    out=t2[:], in0=xm2[:],
                    scalar=g28_c[:, c : c + 1], in1=rstd2_b[:],
                    op0=OP.mult, op1=OP.mult,
                )
                nc.scalar.activation(
                    h_hi[:, c, :], t2[:], AF.Identity,
                    bias=beta8_c[:, c : c + 1],
                )
                nc.gpsimd.tensor_mul(
                    h_hi64[:, c, :], h_hi[:, c, :], c64[:]
                )
                nc.vector.scalar_tensor_tensor(
                    out=h_lo[:, c, :], in0=t2[:],
                    scalar=beta8_c[:, c : c + 1], in1=h_hi[:, c, :],
                    op0=OP.add, op1=OP.subtract,
                )

            # PE keepalive across the LN2/h-prep valley
            ka2 = psc.tile([128, 512], F32, tag="psc", name="ka2")
            for i in range(8):
                nc.tensor.matmul(
                    ka2[:], junk[:, 0:128], junk[:],
                    start=(i == 0), stop=(i == 7),
                )

            # ---------------- FFN (pipelined FFN1 -> FFN2) ----------------
            gT8 = acts.tile([128, FC, SQ], F8, tag="srcT", name="gT8")
            out_sb = acts.tile([128, DC, SQ], F32, tag="kT", name="outsb")
            ff_acc = [
                pbig.tile([128, 4, SQ], F32, tag="pbig", name=f"ffacc{i}")
                for i in range(2)
            ]
            for i in range(2):
                flat = ff_acc[i][:].rearrange("p a q -> p (a q)")
                for half in range(2):
                    nc.tensor.matmul(
                        flat[:, 512 * half : 512 * (half + 1)],
                        zeros16[:], junk[:],
                        start=True, stop=True, skip_group_check=True,
                    )
            for m in range(DC):
                nc.tensor.matmul(
                    ff_acc[m // 4][:, m % 4, :],
                    brow[:, D + 128 * m : D + 128 * (m + 1)], ones_row[:],
                    start=False, stop=False, skip_group_check=True,
                )
            w2hi_tiles = []
            for g in range(4):
                whi = w2hip.tile([128, 4, 2, D], F8, name="w2hit")
                nc.sync.dma_start(
                    out=whi[:], in_=w2hi_d[g].rearrange("c p a n -> p c a n")
                )
                w2hi_tiles.append(whi)
            for g in range(2, 4):
                wlo = w2lop.tile([128, 4, 2, D], F8, name="w2lot")
                nc.sync.dma_start(
                    out=wlo[:], in_=w2lo_d[g].rearrange("c p a n -> p c a n")
                )
                w2lo_tiles[g] = wlo

            for quarter in range(4):
                w1t = w1_tiles[quarter]
                for fi in range(FC // 4):
                    fblk = (FC // 4) * quarter + fi
                    ps = psc.tile([128, SQ], F32, tag="psc", name="psf")
                    for cp in range(DC // 2):
                        nc.tensor.matmul(
                            ps[:],
                            w1t[:, 0, cp, :, 128 * fi : 128 * (fi + 1)],
                            h_hi[:, 2 * cp : 2 * cp + 2, :],
                            start=(cp == 0), stop=False, perf_mode=DR,
                        )
                    for cp in range(DC // 2):
                        nc.tensor.matmul(
                            ps[:],
                            w1t[:, 0, cp, :, 128 * fi : 128 * (fi + 1)],
                            h_lo[:, 2 * cp : 2 * cp + 2, :],
                            start=False, stop=False, perf_mode=DR,
                        )
                    for cp in range(DC // 2):
                        nc.tensor.matmul(
                            ps[:],
                            w1t[:, 1, cp, :, 128 * fi : 128 * (fi + 1)],
                            h_hi64[:, 2 * cp : 2 * cp + 2, :],
                            start=False, stop=(cp == DC // 2 - 1), perf_mode=DR,
                        )
                    sig = smalls.tile([128, SQ], F32, tag="sig", bufs=2, name="sig")
                    nc.scalar.activation(
                        sig[:], ps[:], AF.Sigmoid,
                        bias=b1sig_c[:, fblk : fblk + 1], scale=1.702 / SG,
                    )
                    nc.vector.scalar_tensor_tensor(
                        out=gT8[:, fblk, :], in0=ps[:],
                        scalar=b132_c[:, fblk : fblk + 1], in1=sig[:],
                        op0=OP.add, op1=OP.mult,
                    )
                # FFN2 over the 4 chunks this quarter provides
                whi, wlo = w2hi_tiles[quarter], w2lo_tiles[quarter]
                if quarter < 3:
                    for kk in range(4):
                        k = 4 * quarter + kk
                        for m in range(DC):
                            acc = ff_acc[m // 4][:, m % 4, :]
                            for wt in (whi, wlo):
                                nc.tensor.matmul(
                                    acc,
                                    wt[:, kk, :, 128 * m : 128 * (m + 1)],
                                    gT8[:, 2 * k : 2 * k + 2, :],
                                    start=False, stop=False, perf_mode=DR,
                                    skip_group_check=True,
                                )
                else:
                    # m-outer so each m finishes early; epilogue per pair
                    for m in range(DC):
                        acc = ff_acc[m // 4][:, m % 4, :]
                        for kk in range(4):
                            k = 4 * quarter + kk
                            for wt in (whi, wlo):
                                nc.tensor.matmul(
                                    acc,
                                    wt[:, kk, :, 128 * m : 128 * (m + 1)],
                                    gT8[:, 2 * k : 2 * k + 2, :],
                                    start=False,
                                    stop=(k == FC // 2 - 1 and wt is wlo),
                                    perf_mode=DR,
                                    skip_group_check=True,
                                )
                        nc.vector.scalar_tensor_tensor(
                            out=out_sb[:, m, :], in0=acc,
                            scalar=1.0 / (SG * SW2), in1=x_after[:, m, :],
                            op0=OP.mult, op1=OP.add,
                        )
                        if m % 2 == 1:
                            eng = (nc.sync, nc.scalar, nc.sync, nc.scalar)[m // 2]
                            eng.dma_start(
                                out=out_d[m - 1 : m + 1].rearrange("c p q -> p c q"),
                                in_=out_sb[:, m - 1 : m + 1, :],
                            )

    if not nc.is_finalized():
        nc.finalize()
    _NC_CACHE["nc"] = nc
    return nc


def make_in_maps(inputs):
    src = np.asarray(inputs["src"], dtype=np.float32)
    src_mask = np.asarray(inputs["src_mask"])
    timestep = np.asarray(inputs["timestep"], dtype=np.int32)
    attention_bias = np.asarray(inputs["attention_bias"], dtype=np.float32)

    # host-folded AdaLN table: silu(sin_emb(t)) @ W_ada + b_ada  [100, 2048]
    tbl = (
        _silu_table().astype(np.float64)
        @ np.asarray(inputs["W_ada"], dtype=np.float32).astype(np.float64)
        + np.asarray(inputs["b_ada"], dtype=np.float64)
    ).astype(np.float32).astype(np.float16)

    id8 = np.zeros((128, 2, 128), dtype=np.float32)
    id8[:, 0, :] = np.eye(128) * IDENTV
    id8[:, 1, :] = np.eye(128) * IDENTV

    w1hi, w1lo = _pack_dr_res(inputs["W1"], SW1, 64.0)  # [4cp, 128, 2, F]
    # regroup W1 as [quarter, hi/lo, cp, 128, 2, F//4]
    w1q = np.empty((4, 2, DC // 2, 128, 2, F // 4), dtype=E4)
    for q in range(4):
        w1q[q, 0] = w1hi[:, :, :, (F // 4) * q : (F // 4) * (q + 1)]
        w1q[q, 1] = w1lo[:, :, :, (F // 4) * q : (F // 4) * (q + 1)]
    w2hi, w2lo = _pack_dr_res(inputs["W2"], SW2, 1.0)  # [16cp, 128, 2, D]
    w2hi = np.ascontiguousarray(w2hi.reshape(4, 4, 128, 2, D))
    w2lo = np.ascontiguousarray(w2lo.reshape(4, 4, 128, 2, D))

    consts = np.zeros((128, 128), dtype=np.float32)
    consts[:, 0:8] = _pm(inputs["bq"], DC, 1.0 / 8.0)
    consts[:, 8:16] = _pm(inputs["bk"], DC)
    consts[:, 16:24] = _pm(inputs["bo"], DC, -SX)
    consts[:, 24:32] = _pm(inputs["bo"], DC)
    consts[:, 32:40] = _pm(inputs["b2"], DC)
    consts[:, 40:48] = _pm(inputs["g2"], DC, SH)
    consts[:, 48:56] = _pm(inputs["beta2"], DC, SH)
    consts[:, 56:64] = _pm(inputs["bk"], DC, SX * SW)
    consts[:, 64:96] = _pm(inputs["b1"], FC, 1.702)
    consts[:, 96:128] = _pm(inputs["b1"], FC, SG)

    common = {
        "tbl": tbl,
        "iota100": np.arange(NUM_STEPS, dtype=np.int32).reshape(NUM_STEPS, 1),
        "id8": id8.astype(E4),
        "Wq8": _pack_dr(inputs["Wq"], SW),
        "Wk8": _pack_dr(inputs["Wk"], SW),
        "Wv8": _pack_dr(inputs["Wv"], SW),
        "Wo8": _pack_dr(inputs["Wo"], SW),
        "W18": w1q,
        "W2hi8": w2hi,
        "W2lo8": w2lo,
        "consts_pm": consts,
        "bada_pm": _pm(inputs["b_ada"], 16),
        "bv32_row": (np.asarray(inputs["bv"], dtype=np.float32) * SV)
        .reshape(1, D).astype(np.float16),
        "brow": np.concatenate([
            np.asarray(inputs["bo"], dtype=np.float32) * (SC * SW),
            np.asarray(inputs["b2"], dtype=np.float32) * (SG * SW2),
        ]).reshape(1, 2 * D).astype(np.float16),
    }

    in_maps = []
    for core in range(NC):
        b, j = core // 2, core % 2
        q0, q1 = SQ * j, SQ * (j + 1)
        perm = np.r_[q0:q1, 0:q0, q1:S]
        srcT = np.ascontiguousarray(src[b][perm].T).astype(np.float16).reshape(DC, 128, S)
        # bias [H, SQ, S] -> per head-pair [KB, 128, 2*SQ] (head-interleaved)
        bias_c = attention_bias[b][:, q0:q1, :][:, :, perm]  # [H, SQ, S]
        biasT = np.ascontiguousarray(
            (bias_c.transpose(2, 0, 1) * 8.0)  # [S, H, SQ] scaled
            .reshape(KB, 128, H // 2, 2, SQ)
            .transpose(2, 0, 1, 3, 4)
            .reshape(H // 2, KB, 128, 2 * SQ)
        ).astype(E4)
        mask_c = src_mask[b, 0, q0:q1, :][:, perm]  # [SQ, S]
        maskT = np.ascontiguousarray(
            mask_c.T.astype(np.float32) * MASKV
        ).reshape(KB, 128, SQ).astype(E4)
        m = dict(common)
        m["srcT"] = srcT
        m["biasT"] = biasT
        m["maskT"] = maskT
        m["tstep"] = timestep[b].reshape(1, 1)
        in_maps.append(m)
    return in_maps


def assemble_output(results):
    out = np.empty((B, S, D), dtype=np.float32)
    for core in range(NC):
        b, j = core // 2, core % 2
        o = np.asarray(results[core]["outT"], dtype=np.float32)  # [DC, 128, SQ]
        out[b, SQ * j : SQ * (j + 1), :] = o.reshape(D, SQ).T
    return out


def run(inputs, trace=False, **kw):
    from concourse import bass_utils

    nc = build_nc()
    in_maps = make_in_maps(inputs)
    res = bass_utils.run_bass_kernel_spmd(
        nc, in_maps, list(range(NC)), trace=trace, **kw
    )
    return assemble_output(res.results), res


def kernel(**inputs):
    out, _ = run(inputs)
    return out


# revision 59
# speedup vs baseline: 1.0022x; 1.0022x over previous
"""Trainium2 Bass kernel for nn_Block_15650860827274 (dense transformer block).

Sharding: DP-8 over (batch b, query-half j). Core c = 2*b + j handles batch b
and query positions [256*j, 256*j+256). The sequence axis is rotated on the
host so every core's own queries are columns 0:256 of its (transposed) input;
K/V are computed for the full (permuted) sequence on-device.

Speed levers over the f16 baseline:
- fp8e4 DoubleRow matmuls (0.5 cyc/col, 256-row contraction) for Q/K/V/O
  projections and both FFN layers. FFN precision is recovered with
  same-scale residual compensation: FFN1 runs 3 passes (h_hi@W1hi +
  h_lo@W1hi + (h_hi/64)@W1lo_x64), FFN2 runs 2 passes (g@W2hi + g@W2lo).
- attention bias+mask enter PSUM via fp8 ident-DoubleRow matmuls against
  zero-interleaved buffers (no Pool combine pass, half the moving bytes).
- QK^T stays f16 (64-deep contraction can't pair for DoubleRow).
- probs/V/ctx in fp8; softmax normalization rides the v ones-row.
- DMA split across the SP / Activation / Pool queues (each modeled at
  ~360 GB/s); weights are host-packed fp8 pair-interleaved, fully
  contiguous per partition.
- src shipped f16; AdaLN table (silu(sinemb) @ W_ada + b_ada) folded on
  the host; stats matmuls use f16/f32r moving operands.
- dummy matmuls at t=0 ramp the PE p-state before real work arrives.
"""

import math
import sys

import numpy as np

sys.path.insert(0, "/opt/trn_rl_repo")

import ml_dtypes  # noqa: E402

import concourse.bass as bass  # noqa: E402
import concourse.bacc as bacc  # noqa: E402
import concourse.mybir as mybir  # noqa: E402
from concourse.tile import TileContext  # noqa: E402

F32 = mybir.dt.float32
F32R = mybir.dt.float32r
F16 = mybir.dt.float16
F8 = mybir.dt.float8e4
U8 = mybir.dt.uint8
I32 = mybir.dt.int32
AF = mybir.ActivationFunctionType
OP = mybir.AluOpType
DR = mybir.MatmulPerfMode.DoubleRow
E4 = ml_dtypes.float8_e4m3

B, S, D, H, HD, F = 4, 512, 1024, 16, 64, 4096
SQ = S // 2          # query positions per core
NC = 8               # cores
DC = D // 128        # 8 feature chunks
FC = F // 128        # 32 hidden chunks
KB = S // 128        # 4 key blocks
EPS = 1e-5
NUM_STEPS = 100

# fp8 scales (all powers of two; products must stay under e4m3 max 240)
SX = 8.0             # xT8 = SX * xT
SW = 64.0            # Wq/Wk/Wv/Wo
SV = 32.0            # v8 = SV * (v + bv)
SC = 32.0            # ctx8 = SC * ctx   (via ones-row u = SV/SC = 1)
SH = 8.0             # h_hi8 = SH * h
SW1 = 4.0            # W1hi
SW1L = 256.0         # W1lo stored at SW1*64; moving h_hi/64 compensates
SG = SH * SW1        # gT8 = SG * gelu2(y+b1)  (= 32)
SW2 = 64.0           # W2hi and W2lo (same-scale residual)
MASKV = -160.0       # mask stored as -160*mask at "scale 8" -> -20 in logits
IDENTV = 0.125       # ident-DR stationary value: 0.125 * 8*bias = bias
EXP_SHIFT = math.log(256.0) - 8.0   # probs8 = 256*exp(l-8); factor cancels


def _silu_table():
    half = D // 2
    freqs = np.exp(
        np.arange(half, dtype=np.float32) * np.float32(-math.log(10000.0) / (half - 1))
    ).astype(np.float32)
    t = np.arange(NUM_STEPS, dtype=np.float32)
    x = (t / np.float32(NUM_STEPS) * np.float32(4000.0)).astype(np.float32)
    e = (x[:, None] * freqs[None, :]).astype(np.float32).astype(np.float64)
    emb = np.concatenate([np.sin(e), np.cos(e)], axis=-1)
    silu = emb / (1.0 + np.exp(-emb))
    return silu.astype(np.float32)  # [100, 1024]


def _pm(vec, cols, mul=1.0):
    v = np.asarray(vec, dtype=np.float32) * np.float32(mul)
    return np.ascontiguousarray(v.reshape(cols, 128).T)


def _pack_dr(w, scale):
    """[D_in, N] f32 -> [D_in//256, 128, 2, N] fp8 pair-interleaved."""
    w = np.asarray(w, dtype=np.float32) * np.float32(scale)
    din, n = w.shape
    return np.ascontiguousarray(
        w.reshape(din // 256, 2, 128, n).transpose(0, 2, 1, 3)
    ).astype(E4)


def _pack_dr_res(w, s_hi, s_lo_mul):
    """hi at s_hi, lo = (s_hi*w - hi) at s_hi*s_lo_mul, both DR-packed fp8."""
    w = np.asarray(w, dtype=np.float32)
    hi = (w * np.float32(s_hi)).astype(E4)
    lo = ((w * np.float32(s_hi) - hi.astype(np.float32)) * np.float32(s_lo_mul)).astype(E4)

    def pk(a):
        din, n = a.shape
        return np.ascontiguousarray(
            a.reshape(din // 256, 2, 128, n).transpose(0, 2, 1, 3)
        )

    return pk(hi), pk(lo)


_NC_CACHE = {}


def build_nc():
    if "nc" in _NC_CACHE:
        return _NC_CACHE["nc"]
    nc = bacc.Bacc(
        "TRN2", target_bir_lowering=False, debug=False, num_devices=NC
    )

    # ---- I/O ----
    srcT_d = nc.dram_tensor("srcT", [DC, 128, S], F16, kind="ExternalInput")
    # bias: per head-pair hp: [KB, 128, 2*SQ] with the two heads' q-rows
    # interleaved so DMA elems are 512B
    biasT_d = nc.dram_tensor("biasT", [H // 2, KB, 128, 2 * SQ], F8, kind="ExternalInput")
    maskT_d = nc.dram_tensor("maskT", [KB, 128, SQ], F8, kind="ExternalInput")
    tstep_d = nc.dram_tensor("tstep", [1, 1], I32, kind="ExternalInput")
    tbl_d = nc.dram_tensor("tbl", [NUM_STEPS, 2 * D], F16, kind="ExternalInput")
    iota_d = nc.dram_tensor("iota100", [NUM_STEPS, 1], I32, kind="ExternalInput")
    id8_d = nc.dram_tensor("id8", [128, 2, 128], F8, kind="ExternalInput")
    wq_d = nc.dram_tensor("Wq8", [DC // 2, 128, 2, D], F8, kind="ExternalInput")
    wk_d = nc.dram_tensor("Wk8", [DC // 2, 128, 2, D], F8, kind="ExternalInput")
    wv_d = nc.dram_tensor("Wv8", [DC // 2, 128, 2, D], F8, kind="ExternalInput")
    wo_d = nc.dram_tensor("Wo8", [DC // 2, 128, 2, D], F8, kind="ExternalInput")
    # W1: [quarter, hi/lo, cp, 128, 2, F//4]
    w1_d = nc.dram_tensor("W18", [4, 2, DC // 2, 128, 2, F // 4], F8, kind="ExternalInput")
    # W2: [group, cp-in-group, 128, 2, D] split into hi and lo tensors
    w2hi_d = nc.dram_tensor("W2hi8", [4, 4, 128, 2, D], F8, kind="ExternalInput")
    w2lo_d = nc.dram_tensor("W2lo8", [4, 4, 128, 2, D], F8, kind="ExternalInput")
    # packed per-partition consts [128, 128] f32:
    # cols 0:8 bq/8 | 8:16 bk | 16:24 -8*bo | 24:32 bo | 32:40 b2 | 40:48 8*g2
    # | 48:56 8*beta2 | 56:64 unused | 64:96 1.702*b1 | 96:128 32*b1
    consts_d = nc.dram_tensor("consts_pm", [128, 128], F32, kind="ExternalInput")
    bada_d = nc.dram_tensor("bada_pm", [128, 16], F32, kind="ExternalInput")
    brow_d = nc.dram_tensor("brow", [1, 2 * D], F16, kind="ExternalInput")
    bv_d = nc.dram_tensor("bv32_row", [1, D], F16, kind="ExternalInput")
    out_d = nc.dram_tensor("outT", [DC, 128, SQ], F32, kind="ExternalOutput")

    with TileContext(nc) as tc:
        with (
            nc.allow_low_precision(reason="fp8/f16 paths are error-analyzed"),
            tc.tile_pool(name="consts", bufs=1) as cpool,
            tc.tile_pool(name="acts", bufs=1) as acts,
            tc.tile_pool(name="wqkvo", bufs=3) as wqkvo,
            tc.tile_pool(name="w1p", bufs=3) as w1p,
            tc.tile_pool(name="w2hip", bufs=2) as w2hip,
            tc.tile_pool(name="w2lop", bufs=2) as w2lop,
            tc.tile_pool(name="probs", bufs=2) as probsp,
            tc.tile_pool(name="smalls", bufs=3) as smalls,
            tc.tile_pool(name="st", bufs=3) as stp,
            tc.tile_pool(name="stb", bufs=2) as stbp,
            tc.tile_pool(name="ovl1", bufs=1) as ovl1,
            tc.tile_pool(name="ovl2", bufs=1) as ovl2,
            tc.tile_pool(name="pstat", bufs=2, space="PSUM") as pstat,
            tc.tile_pool(name="pbig", bufs=2, space="PSUM") as pbig,
            tc.tile_pool(name="psc", bufs=2, space="PSUM") as psc,
        ):
            # ---------------- consts + PE warmup ----------------
            ones_h = cpool.tile([128, 1], F16, tag="onesh")
            nc.vector.memset(ones_h[:], 1.0)
            cshift = cpool.tile([128, 1], F32, tag="cshift")
            nc.vector.memset(cshift[:], EXP_SHIFT)
            epsc = cpool.tile([1, 1], F32, tag="epsc")
            nc.vector.memset(epsc[:], EPS)
            warm0 = cpool.tile([1, 4], F32, tag="warm0")
            nc.scalar.activation(warm0[:, 0:1], epsc[:], AF.Sqrt)
            junk = cpool.tile([128, 512], F16, tag="junk")
            nc.vector.memset(junk[:], 0.001)
            zeros16 = cpool.tile([128, 128], F16, tag="zeros16")
            nc.vector.memset(zeros16[:], 0.0)
            warm_ps = psc.tile([128, 512], F32, tag="psc", name="warmps")
            for i in range(6):
                nc.tensor.matmul(
                    warm_ps[:], junk[:, 0:128], junk[:],
                    start=(i == 0), stop=(i == 5),
                )

            # ---------------- SP queue: src first, then weights ----------------
            srcT = acts.tile([128, DC, S], F16, tag="srcT")
            for hh in range(2):
                nc.sync.dma_start(
                    out=srcT[:, 4 * hh : 4 * (hh + 1), :],
                    in_=srcT_d[4 * hh : 4 * (hh + 1)].rearrange("c p s -> p c s"),
                )
            wq_t = wqkvo.tile([128, DC // 2, 2, D], F8, name="wqkvot")
            nc.sync.dma_start(out=wq_t[:], in_=wq_d[:].rearrange("c p a n -> p c a n"))
            wk_t = wqkvo.tile([128, DC // 2, 2, D], F8, name="wqkvot")
            nc.sync.dma_start(out=wk_t[:], in_=wk_d[:].rearrange("c p a n -> p c a n"))
            wv_t = wqkvo.tile([128, DC // 2, 2, D], F8, name="wqkvot")
            nc.sync.dma_start(out=wv_t[:], in_=wv_d[:].rearrange("c p a n -> p c a n"))
            wo_t = wqkvo.tile([128, DC // 2, 2, D], F8, name="wqkvot")
            nc.sync.dma_start(out=wo_t[:], in_=wo_d[:].rearrange("c p a n -> p c a n"))

            # ---------------- ACT queue: small loads ----------------
            t_sb = cpool.tile([1, 1], I32, tag="tsb")
            nc.scalar.dma_start(out=t_sb[:], in_=tstep_d[:])
            iota_pm = cpool.tile([NUM_STEPS, 1], I32, tag="iota")
            nc.scalar.dma_start(out=iota_pm[:], in_=iota_d[:])
            tbl_sb = cpool.tile([NUM_STEPS, 2 * D], F16, tag="tbl")
            nc.scalar.dma_start(out=tbl_sb[:], in_=tbl_d[:])
            consts_sb = cpool.tile([128, 128], F32, tag="consts")
            nc.scalar.dma_start(out=consts_sb[:], in_=consts_d[:])
            bada_sb = cpool.tile([128, 16], F32, tag="bada")
            nc.scalar.dma_start(out=bada_sb[:], in_=bada_d[:])
            id8 = cpool.tile([128, 2, 128], F8, tag="id8")
            nc.scalar.dma_start(out=id8[:], in_=id8_d[:])
            bv_row = stp.tile([1, D], F16, tag="st", name="bvrow")
            nc.scalar.dma_start(out=bv_row[:], in_=bv_d[:])
            brow = cpool.tile([1, 2 * D], F16, tag="brow")
            nc.scalar.dma_start(out=brow[:], in_=brow_d[:])
            warm = cpool.tile([1, 4], F32, tag="warm")
            ones_row = cpool.tile([1, SQ], F16, tag="onesrow")
            nc.vector.memset(ones_row[:], 1.0)
            c64 = cpool.tile([128, SQ], F16, tag="c64")
            nc.vector.memset(c64[:], 1.0 / 64.0)


            bq8_c = consts_sb[:, 0:8]
            bk_c = consts_sb[:, 8:16]
            bon8_c = consts_sb[:, 16:24]
            bo_c = consts_sb[:, 24:32]
            b2_c = consts_sb[:, 32:40]
            g28_c = consts_sb[:, 40:48]
            beta8_c = consts_sb[:, 48:56]
            b1sig_c = consts_sb[:, 64:96]
            b132_c = consts_sb[:, 96:128]
            bk512_c = consts_sb[:, 56:64]
            recip512 = cpool.tile([128, S], F8, tag="recip512")
            nc.vector.memset(recip512[:], 1.0 / (SX * SW))

            # ---------------- Pool queue: mask, bias pairs (W2lo later) -------
            maskz = cpool.tile([128, 2, KB, SQ], F8, tag="maskz")
            nc.vector.memset(maskz[:, 1, :, :], 0.0)
            nc.gpsimd.dma_start(
                out=maskz[:, 0, :, :], in_=maskT_d[:].rearrange("a p q -> p a q")
            )
            # combined bias buffer, planes [dataA, zeros, dataB]: the DR ident
            # trick reads (data, zero) or (zero, data) pairs; id8 has the
            # ident in both pair slots so order doesn't matter.
            bias3 = cpool.tile([128, 3, 2, KB, 2, SQ], F8, tag="bias3")
            nc.gpsimd.memset(bias3[:, 1, 0], 0.0)
            nc.gpsimd.memset(bias3[:, 1, 1], 0.0)

            t_b = cpool.tile([NUM_STEPS, 1], I32, tag="tb")
            nc.gpsimd.partition_broadcast(t_b[:], t_sb[:])
            onehot = cpool.tile([NUM_STEPS, 1], F16, tag="onehot")
            nc.vector.tensor_tensor(
                out=onehot[:], in0=iota_pm[:], in1=t_b[:], op=OP.is_equal
            )

            def bias_dma(g):
                nc.gpsimd.dma_start(
                    out=bias3[:, 2 * (g % 2)].rearrange("p h a b q -> p (h a b q)"),
                    in_=biasT_d[2 * g : 2 * g + 2].rearrange(
                        "h a p q -> p (h a) q"
                    ),
                )

            bias_dma(0)

            src2 = acts.tile([128, DC, S], F16, tag="kT", name="src2")
            for c in range(DC):
                nc.vector.tensor_mul(src2[:, c, :], srcT[:, c, :], srcT[:, c, :])
            sum_x = pstat.tile([1, S], F32, tag="pstat")
            for c in range(DC):
                nc.tensor.matmul(
                    sum_x[:], ones_h[:], srcT[:, c, :],
                    start=(c == 0), stop=(c == DC - 1),
                )
            sum_x2 = pstat.tile([1, S], F32, tag="pstat")
            for c in range(DC):
                nc.tensor.matmul(
                    sum_x2[:], ones_h[:], src2[:, c, :],
                    start=(c == 0), stop=(c == DC - 1),
                )
            # ---------------- timestep embedding (host-folded table) ---------
            emb_ps = psc.tile([128, 16], F32, tag="psc", name="embps")
            for i in range(16):
                nc.tensor.matmul(
                    emb_ps[:, i : i + 1],
                    tbl_sb[:, 128 * i : 128 * (i + 1)],
                    onehot[:],
                    start=True, stop=True,
                )
            ss_pm = cpool.tile([128, 16], F32, tag="sspm")
            nc.vector.tensor_add(ss_pm[:], emb_ps[:], bada_sb[:])
            scale1p = cpool.tile([128, DC], F32, tag="scale1p")
            nc.vector.tensor_scalar_add(scale1p[:], ss_pm[:, 0:DC], 1.0)
            scale1p8 = cpool.tile([128, DC], F32, tag="scale1p8")
            nc.vector.tensor_scalar_mul(scale1p8[:], scale1p[:], SX)
            shift8 = cpool.tile([128, DC], F32, tag="shift8")
            nc.vector.tensor_scalar_mul(shift8[:], ss_pm[:, DC:16], SX)

            # ---------------- LN1 stats ----------------
            mean1h = stp.tile([1, S], F16, tag="st", name="mean1h")
            nc.vector.tensor_scalar_mul(mean1h[:], sum_x[:], 1.0 / D)
            var1 = stp.tile([1, S], F32, tag="st")
            nc.vector.tensor_mul(var1[:], mean1h[:], mean1h[:])
            nc.vector.scalar_tensor_tensor(
                out=var1[:], in0=sum_x2[:], scalar=1.0 / D, in1=var1[:],
                op0=OP.mult, op1=OP.subtract,
            )
            sd1 = stp.tile([1, S], F32, tag="st", name="sd1")
            nc.scalar.activation(sd1[:], var1[:], AF.Sqrt, bias=epsc[:])
            rstd1 = stp.tile([1, S], F16, tag="st", name="rstd1")
            nc.vector.reciprocal(rstd1[:], sd1[:])
            mean1_b = stbp.tile([128, S], F16, tag="stb")
            nc.gpsimd.partition_broadcast(mean1_b[:], mean1h[:])
            rstd1_b = stbp.tile([128, S], F16, tag="stb")
            nc.gpsimd.partition_broadcast(rstd1_b[:], rstd1[:])

            # PE keepalive while DVE builds xT
            ka_ps = psc.tile([128, 512], F32, tag="psc", name="kaps")
            for i in range(16):
                nc.tensor.matmul(
                    ka_ps[:], junk[:, 0:128], junk[:],
                    start=(i == 0), stop=(i == 15),
                )

            # ---------------- xT ----------------
            # half0 (queries): xT f32 (with +bo fold = xTbo) and xT8
            # half1 (other keys): xT8 only
            xTbo = acts.tile([128, DC, SQ], F16, tag="xTbo")
            xT8a = acts.tile([128, DC, SQ], F8, tag="xT8a")
            xT8b = acts.tile([128, DC, SQ], F8, tag="xT8b")
            sl0 = slice(0, SQ)
            sl1 = slice(SQ, S)
            t0s = {}
            for c in range(DC):
                xm = smalls.tile([128, SQ], F16, tag="xm", bufs=4, name="xm0")
                nc.gpsimd.tensor_sub(xm[:], srcT[:, c, sl0], mean1_b[:, sl0])
                t0 = smalls.tile([128, SQ], F16, tag="t0", bufs=4, name="t0")
                nc.vector.scalar_tensor_tensor(
                    out=t0[:], in0=xm[:],
                    scalar=scale1p[:, c : c + 1], in1=rstd1_b[:, sl0],
                    op0=OP.mult, op1=OP.mult,
                )
                nc.scalar.activation(
                    xT8a[:, c, :], t0[:], AF.Identity,
                    bias=shift8[:, c : c + 1], scale=SX,
                )
                t0s[c] = t0
                if c >= 3:
                    cc = c - 3
                    nc.vector.tensor_scalar_add(
                        xTbo[:, cc, :], t0s.pop(cc)[:],
                        ss_pm[:, DC + cc : DC + cc + 1],
                    )
            for cc in sorted(t0s):
                nc.vector.tensor_scalar_add(
                    xTbo[:, cc, :], t0s[cc][:], ss_pm[:, DC + cc : DC + cc + 1]
                )
            for c in range(DC):
                xm = smalls.tile([128, SQ], F16, tag="xm", bufs=4, name="xm1")
                nc.gpsimd.tensor_sub(xm[:], srcT[:, c, sl1], mean1_b[:, sl1])
                t8 = smalls.tile([128, SQ], F32, tag="t8", bufs=4, name="t8")
                nc.vector.scalar_tensor_tensor(
                    out=t8[:], in0=xm[:],
                    scalar=scale1p8[:, c : c + 1], in1=rstd1_b[:, sl1],
                    op0=OP.mult, op1=OP.mult,
                )
                nc.vector.tensor_scalar_add(
                    xT8b[:, c, :], t8[:], shift8[:, c : c + 1]
                )

            # ---------------- Q, K projections (DR) ----------------
            qT = acts.tile([128, DC, SQ], F16, tag="qT")
            for m in range(DC):
                ps = pbig.tile([128, 4, SQ], F32, tag="pbig", name="psq")
                pq = ps[:, 0, :]
                for cp in range(DC // 2):
                    nc.tensor.matmul(
                        pq,
                        wq_t[:, cp, :, 128 * m : 128 * (m + 1)],
                        xT8a[:, 2 * cp : 2 * cp + 2, :],
                        start=(cp == 0), stop=(cp == DC // 2 - 1),
                        perf_mode=DR,
                    )
                nc.scalar.activation(
                    qT[:, m, :], pq, AF.Identity,
                    bias=bq8_c[:, m : m + 1], scale=1.0 / (SX * SW * 8.0),
                )
            kT = acts.tile([128, DC, S], F16, tag="kT")
            for m in range(DC):
                ps = pbig.tile([128, 4, SQ], F32, tag="pbig", name="psk")
                pk = ps[:].rearrange("p a q -> p (a q)")[:, 0:S]
                for xh, half in ((xT8a, 0), (xT8b, 1)):
                    for cp in range(DC // 2):
                        nc.tensor.matmul(
                            pk[:, SQ * half : SQ * (half + 1)],
                            wk_t[:, cp, :, 128 * m : 128 * (m + 1)],
                            xh[:, 2 * cp : 2 * cp + 2, :],
                            start=(cp == 0), stop=(cp == DC // 2 - 1),
                            perf_mode=DR, skip_group_check=True,
                        )
                if m % 2 == 0:
                    nc.scalar.activation(
                        kT[:, m, :], pk, AF.Identity,
                        bias=bk_c[:, m : m + 1], scale=1.0 / (SX * SW),
                    )
                else:
                    nc.vector.scalar_tensor_tensor(
                        out=kT[:, m, :], in0=pk,
                        scalar=bk512_c[:, m : m + 1], in1=recip512[:],
                        op0=OP.add, op1=OP.mult,
                    )

            bv32_b = cpool.tile([128, D], F16, tag="bvb")
            nc.gpsimd.partition_broadcast(bv32_b[:], bv_row[:])
            w2lo_tiles = [None] * 4
            for g in range(2):
                wlo = w2lop.tile([128, 4, 2, D], F8, name="w2lot")
                nc.sync.dma_start(
                    out=wlo[:], in_=w2lo_d[g].rearrange("c p a n -> p c a n")
                )
                w2lo_tiles[g] = wlo

            # ---------------- V projection (token-major, DR) ----------------
            v_sb = acts.tile([128, KB, H, HD + 1], F8, tag="v")
            nc.vector.memset(v_sb[:, :, :, HD : HD + 1], 1.0)  # SV/SC = 1
            for t in range(KB):
                for half in range(2):
                    ps = pbig.tile([128, 4, SQ], F32, tag="pbig", name="psv")
                    pv_ = ps[:].rearrange("p a q -> p (a q)")[:, 0:512]
                    xh = xT8a if t < 2 else xT8b
                    tt = t % 2
                    for cp in range(DC // 2):
                        nc.tensor.matmul(
                            pv_,
                            xh[:, 2 * cp : 2 * cp + 2, 128 * tt : 128 * (tt + 1)],
                            wv_t[:, cp, :, 512 * half : 512 * (half + 1)],
                            start=(cp == 0), stop=(cp == DC // 2 - 1),
                            perf_mode=DR,
                        )
                    ev = nc.vector
                    ev.scalar_tensor_tensor(
                        out=v_sb[:, t, 8 * half : 8 * (half + 1), 0:HD],
                        in0=pv_.rearrange("p (h d) -> p h d", h=8),
                        scalar=SV / (SX * SW),
                        in1=bv32_b[:, 512 * half : 512 * (half + 1)].rearrange(
                            "p (h d) -> p h d", h=8
                        ),
                        op0=OP.mult, op1=OP.add,
                    )

            # ---------------- attention, per head ----------------
            ctx8 = acts.tile([128, DC, SQ], F8, tag="ctx8")
            for h in range(H):
                hc, hr = h // 2, 64 * (h % 2)
                if h == 0:
                    bias_dma(1)
                if h in (4, 8):
                    bias_dma(h // 4 + 1)
                z = (h // 4) % 2  # buffer A -> planes 0:2, buffer B -> 1:3
                bias_mv = bias3[:, z : z + 2, (h // 2) % 2, :, h % 2, :]
                probs = probsp.tile([128, KB, SQ], F8, tag="probs")
                sc = pbig.tile([128, KB, SQ], F32, tag="pbig", name="scps")
                for kc in range(KB):
                    nc.tensor.matmul(
                        sc[:, kc, :],
                        id8[:],
                        bias_mv[:, :, kc, :],
                        start=True, stop=False, perf_mode=DR,
                    )
                    nc.tensor.matmul(
                        sc[:, kc, :],
                        id8[:],
                        maskz[:, :, kc, :],
                        start=False, stop=False, perf_mode=DR,
                    )
                    nc.tensor.matmul(
                        sc[:, kc, :],
                        kT[hr : hr + 64, hc, 128 * kc : 128 * (kc + 1)],
                        qT[hr : hr + 64, hc, :],
                        start=False, stop=True,
                    )
                nc.scalar.activation(
                    probs[:].rearrange("p a q -> p (a q)"),
                    sc[:].rearrange("p a q -> p (a q)"),
                    AF.Exp, bias=cshift[:],
                )
                cps = psc.tile([128, SQ], F32, tag="psc", name="cps")[: HD + 1]
                for jp in range(KB // 2):
                    nc.tensor.matmul(
                        cps,
                        v_sb[:, 2 * jp : 2 * jp + 2, h, :],
                        probs[:, 2 * jp : 2 * jp + 2, :],
                        start=(jp == 0), stop=(jp == KB // 2 - 1),
                        perf_mode=DR,
                    )
                rh = smalls.tile([1, SQ], F32, tag="rh", bufs=2)
                nc.vector.reciprocal(rh[:], cps[HD : HD + 1, :])
                last_rh = rh
                rh_b = smalls.tile([64, SQ], F32, tag="rhb", bufs=2)
                nc.gpsimd.partition_broadcast(rh_b[:], rh[:])
                nc.vector.tensor_mul(
                    ctx8[hr : hr + 64, hc, :], cps[0:HD, :], rh_b[:]
                )

            nc.scalar.activation(warm[:, 1:2], last_rh[0:1, 0:1], AF.Sqrt)

            # SP queue: W1 quarters (2MB each: hi+lo)
            w1_tiles = []
            for q in range(4):
                wt = w1p.tile([128, 2, DC // 2, 2, F // 4], F8, name="w1t")
                nc.sync.dma_start(
                    out=wt[:], in_=w1_d[q].rearrange("l c p a n -> p l c a n")
                )
                w1_tiles.append(wt)

            # ---------------- out projection + residual ----------------
            x_after = acts.tile([128, DC, SQ], F16, tag="xaf")
            xsq = acts.tile([128, DC, SQ], F16, tag="srcT", name="xsq")
            for m in range(DC):
                ps = pbig.tile([128, 4, SQ], F32, tag="pbig", name="pso")
                po = ps[:, 0, :]
                for cp in range(DC // 2):
                    nc.tensor.matmul(
                        po,
                        wo_t[:, cp, :, 128 * m : 128 * (m + 1)],
                        ctx8[:, 2 * cp : 2 * cp + 2, :],
                        start=(cp == 0), stop=False,
                        perf_mode=DR,
                    )
                nc.tensor.matmul(
                    po, brow[:, 128 * m : 128 * (m + 1)], ones_row[:],
                    start=False, stop=True,
                )
                nc.vector.scalar_tensor_tensor(
                    out=x_after[:, m, :], in0=po, scalar=1.0 / (SC * SW),
                    in1=xTbo[:, m, :], op0=OP.mult, op1=OP.add,
                )
                nc.gpsimd.tensor_mul(
                    xsq[:, m, :], x_after[:, m, :], x_after[:, m, :]
                )

            # ---------------- LN2 + h quantization ----------------
            sum2_x = pstat.tile([1, S], F32, tag="pstat", name="s2x")[:, :SQ]
            for c in range(DC):
                nc.tensor.matmul(
                    sum2_x, ones_h[:], x_after[:, c, :],
                    start=(c == 0), stop=(c == DC - 1),
                )
            sum2_x2 = pstat.tile([1, S], F32, tag="pstat", name="s2x2")[:, :SQ]
            for c in range(DC):
                nc.tensor.matmul(
                    sum2_x2, ones_h[:], xsq[:, c, :],
                    start=(c == 0), stop=(c == DC - 1),
                )
            mean2h = stp.tile([1, SQ], F16, tag="st", name="mean2h")
            nc.vector.tensor_scalar_mul(mean2h[:], sum2_x, 1.0 / D)
            var2 = stp.tile([1, SQ], F32, tag="st")
            nc.vector.tensor_mul(var2[:], mean2h[:], mean2h[:])
            nc.vector.scalar_tensor_tensor(
                out=var2[:], in0=sum2_x2, scalar=1.0 / D, in1=var2[:],
                op0=OP.mult, op1=OP.subtract,
            )
            sd2 = stp.tile([1, SQ], F32, tag="st", name="sd2")
            nc.scalar.activation(sd2[:], var2[:], AF.Sqrt, bias=epsc[:])
            rstd2 = stp.tile([1, SQ], F16, tag="st", name="rstd2")
            nc.vector.reciprocal(rstd2[:], sd2[:])
            nc.scalar.activation(warm[:, 2:3], rstd2[0:1, 0:1], AF.Sigmoid)
            mean2_b = stbp.tile([128, SQ], F16, tag="stb2", bufs=1)
            nc.gpsimd.partition_broadcast(mean2_b[:], mean2h[:])
            rstd2_b = stbp.tile([128, SQ], F16, tag="stb2b", bufs=1)
            nc.gpsimd.partition_broadcast(rstd2_b[:], rstd2[:])

            h_hi = acts.tile([128, DC, SQ], F8, tag="hhi")
            h_hi64 = acts.tile([128, DC, SQ], F8, tag="hhi64")
            h_lo = acts.tile([128, DC, SQ], F8, tag="hlo")
            for c in range(DC):
                xm2 = smalls.tile([128, SQ], F16, tag="xm", bufs=4, name="xm2")
                nc.gpsimd.tensor_sub(xm2[:], x_after[:, c, :], mean2_b[:])
                t2 = smalls.tile([128, SQ], F32, tag="t8", bufs=4, name="t2")
                nc.vector.scalar_tensor_tensor(
                    out=t2[:], in0=xm2[:],
                    scalar=g28_c[:, c : c + 1], in1=rstd2_b[:],
                    op0=OP.mult, op1=OP.mult,
                )
                nc.scalar.activation(
                    h_hi[:, c, :], t2[:], AF.Identity,
                    bias=beta8_c[:, c : c + 1],
                )
                nc.gpsimd.tensor_mul(
                    h_hi64[:, c, :], h_hi[:, c, :], c64[:]
                )
                nc.vector.scalar_tensor_tensor(
                    out=h_lo[:, c, :], in0=t2[:],
                    scalar=beta8_c[:, c : c + 1], in1=h_hi[:, c, :],
                    op0=OP.add, op1=OP.subtract,
                )

            # PE keepalive across the LN2/h-prep valley
            ka2 = psc.tile([128, 512], F32, tag="psc", name="ka2")
            for i in range(8):
                nc.tensor.matmul(
                    ka2[:], junk[:, 0:128], junk[:],
                    start=(i == 0), stop=(i == 7),
                )

            # ---------------- FFN (pipelined FFN1 -> FFN2) ----------------
            gT8 = acts.tile([128, FC, SQ], F8, tag="srcT", name="gT8")
            out_sb = acts.tile([128, DC, SQ], F32, tag="kT", name="outsb")
            ff_acc = [
                pbig.tile([128, 4, SQ], F32, tag="pbig", name=f"ffacc{i}")
                for i in range(2)
            ]
            for i in range(2):
                flat = ff_acc[i][:].rearrange("p a q -> p (a q)")
                for half in range(2):
                    nc.tensor.matmul(
                        flat[:, 512 * half : 512 * (half + 1)],
                        zeros16[:], junk[:],
                        start=True, stop=True, skip_group_check=True,
                    )
            for m in range(DC):
                nc.tensor.matmul(
                    ff_acc[m // 4][:, m % 4, :],
                    brow[:, D + 128 * m : D + 128 * (m + 1)], ones_row[:],
                    start=False, stop=False, skip_group_check=True,
                )
            w2hi_tiles = []
            for g in range(4):
                whi = w2hip.tile([128, 4, 2, D], F8, name="w2hit")
                nc.sync.dma_start(
                    out=whi[:], in_=w2hi_d[g].rearrange("c p a n -> p c a n")
                )
                w2hi_tiles.append(whi)
            for g in range(2, 4):
                wlo = w2lop.tile([128, 4, 2, D], F8, name="w2lot")
                nc.sync.dma_start(
                    out=wlo[:], in_=w2lo_d[g].rearrange("c p a n -> p c a n")
                )
                w2lo_tiles[g] = wlo

            for quarter in range(4):
                w1t = w1_tiles[quarter]
                for fi in range(FC // 4):
                    fblk = (FC // 4) * quarter + fi
                    ps = psc.tile([128, SQ], F32, tag="psc", name="psf")
                    for cp in range(DC // 2):
                        nc.tensor.matmul(
                            ps[:],
                            w1t[:, 0, cp, :, 128 * fi : 128 * (fi + 1)],
                            h_hi[:, 2 * cp : 2 * cp + 2, :],
                            start=(cp == 0), stop=False, perf_mode=DR,
                        )
                    for cp in range(DC // 2):
                        nc.tensor.matmul(
                            ps[:],
                            w1t[:, 0, cp, :, 128 * fi : 128 * (fi + 1)],
                            h_lo[:, 2 * cp : 2 * cp + 2, :],
                            start=False, stop=False, perf_mode=DR,
                        )
                    for cp in range(DC // 2):
                        nc.tensor.matmul(
                            ps[:],
                            w1t[:, 1, cp, :, 128 * fi : 128 * (fi + 1)],
                            h_hi64[:, 2 * cp : 2 * cp + 2, :],
                            start=False, stop=(cp == DC // 2 - 1), perf_mode=DR,
                        )
                    sig = smalls.tile([128, SQ], F32, tag="sig", bufs=2, name="sig")
                    nc.scalar.activation(
                        sig[:], ps[:], AF.Sigmoid,
                        bias=b1sig_c[:, fblk : fblk + 1], scale=1.702 / SG,
                    )
                    nc.vector.scalar_tensor_tensor(
                        out=gT8[:, fblk, :], in0=ps[:],
                        scalar=b132_c[:, fblk : fblk + 1], in1=sig[:],
                        op0=OP.add, op1=OP.mult,
                    )
                # FFN2 over the 4 chunks this quarter provides
                whi, wlo = w2hi_tiles[quarter], w2lo_tiles[quarter]
                if quarter < 3:
                    for kk in range(4):
                        k = 4 * quarter + kk
                        for m in range(DC):
                            acc = ff_acc[m // 4][:, m % 4, :]
                            for wt in (whi, wlo):
                                nc.tensor.matmul(
                                    acc,
                                    wt[:, kk, :, 128 * m : 128 * (m + 1)],
                                    gT8[:, 2 * k : 2 * k + 2, :],
                                    start=False, stop=False, perf_mode=DR,
                                    skip_group_check=True,
                                )
                else:
                    # m-outer so each m finishes early; epilogue per pair
                    for m in range(DC):
                        acc = ff_acc[m // 4][:, m % 4, :]
                        for kk in range(4):
                            k = 4 * quarter + kk
                            for wt in (whi, wlo):
                                nc.tensor.matmul(
                                    acc,
                                    wt[:, kk, :, 128 * m : 128 * (m + 1)],
                                    gT8[:, 2 * k : 2 * k + 2, :],
                                    start=False,
                                    stop=(k == FC // 2 - 1 and wt is wlo),
                                    perf_mode=DR,
                                    skip_group_check=True,
                                )
                        nc.vector.scalar_tensor_tensor(
                            out=out_sb[:, m, :], in0=acc,
                            scalar=1.0 / (SG * SW2), in1=x_after[:, m, :],
                            op0=OP.mult, op1=OP.add,
                        )
                        if m % 2 == 1:
                            eng = (nc.sync, nc.scalar, nc.sync, nc.scalar)[m // 2]
                            eng.dma_start(
                                out=out_d[m - 1 : m + 1].rearrange("c p q -> p c q"),
                                in_=out_sb[:, m - 1 : m + 1, :],
                            )

    if not nc.is_finalized():
        nc.finalize()
    _NC_CACHE["nc"] = nc
    return nc


def make_in_maps(inputs):
    src = np.asarray(inputs["src"], dtype=np.float32)
    src_mask = np.asarray(inputs["src_mask"])
    timestep = np.asarray(inputs["timestep"], dtype=np.int32)
    attention_bias = np.asarray(inputs["attention_bias"], dtype=np.float32)

    # host-folded AdaLN table: silu(sin_emb(t)) @ W_ada + b_ada  [100, 2048]
    tbl = (
        _silu_table().astype(np.float64)
        @ np.asarray(inputs["W_ada"], dtype=np.float32).astype(np.float64)
        + np.asarray(inputs["b_ada"], dtype=np.float64)
    ).astype(np.float32).astype(np.float16)

    id8 = np.zeros((128, 2, 128), dtype=np.float32)
    id8[:, 0, :] = np.eye(128) * IDENTV
    id8[:, 1, :] = np.eye(128) * IDENTV

    w1hi, w1lo = _pack_dr_res(inputs["W1"], SW1, 64.0)  # [4cp, 128, 2, F]
    # regroup W1 as [quarter, hi/lo, cp, 128, 2, F//4]
    w1q = np.empty((4, 2, DC // 2, 128, 2, F // 4), dtype=E4)
    for q in range(4):
        w1q[q, 0] = w1hi[:, :, :, (F // 4) * q : (F // 4) * (q + 1)]
        w1q[q, 1] = w1lo[:, :, :, (F // 4) * q : (F // 4) * (q + 1)]
    w2hi, w2lo = _pack_dr_res(inputs["W2"], SW2, 1.0)  # [16cp, 128, 2, D]
    w2hi = np.ascontiguousarray(w2hi.reshape(4, 4, 128, 2, D))
    w2lo = np.ascontiguousarray(w2lo.reshape(4, 4, 128, 2, D))

    consts = np.zeros((128, 128), dtype=np.float32)
    consts[:, 0:8] = _pm(inputs["bq"], DC, 1.0 / 8.0)
    consts[:, 8:16] = _pm(inputs["bk"], DC)
    consts[:, 16:24] = _pm(inputs["bo"], DC, -SX)
    consts[:, 24:32] = _pm(inputs["bo"], DC)
    consts[:, 32:40] = _pm(inputs["b2"], DC)
    consts[:, 40:48] = _pm(inputs["g2"], DC, SH)
    consts[:, 48:56] = _pm(inputs["beta2"], DC, SH)
    consts[:, 56:64] = _pm(inputs["bk"], DC, SX * SW)
    consts[:, 64:96] = _pm(inputs["b1"], FC, 1.702)
    consts[:, 96:128] = _pm(inputs["b1"], FC, SG)

    common = {
        "tbl": tbl,
        "iota100": np.arange(NUM_STEPS, dtype=np.int32).reshape(NUM_STEPS, 1),
        "id8": id8.astype(E4),
        "Wq8": _pack_dr(inputs["Wq"], SW),
        "Wk8": _pack_dr(inputs["Wk"], SW),
        "Wv8": _pack_dr(inputs["Wv"], SW),
        "Wo8": _pack_dr(inputs["Wo"], SW),
        "W18": w1q,
        "W2hi8": w2hi,
        "W2lo8": w2lo,
        "consts_pm": consts,
        "bada_pm": _pm(inputs["b_ada"], 16),
        "bv32_row": (np.asarray(inputs["bv"], dtype=np.float32) * SV)
        .reshape(1, D).astype(np.float16),
        "brow": np.concatenate([
            np.asarray(inputs["bo"], dtype=np.float32) * (SC * SW),
            np.asarray(inputs["b2"], dtype=np.float32) * (SG * SW2),
        ]).reshape(1, 2 * D).astype(np.float16),
    }

    in_maps = []
    for core in range(NC):
        b, j = core // 2, core % 2
        q0, q1 = SQ * j, SQ * (j + 1)
        perm = np.r_[q0:q1, 0:q0, q1:S]
        srcT = np.ascontiguousarray(src[b][perm].T).astype(np.float16).reshape(DC, 128, S)
        # bias [H, SQ, S] -> per head-pair [KB, 128, 2*SQ] (head-interleaved)
        bias_c = attention_bias[b][:, q0:q1, :][:, :, perm]  # [H, SQ, S]
        biasT = np.ascontiguousarray(
            (bias_c.transpose(2, 0, 1) * 8.0)  # [S, H, SQ] scaled
            .reshape(KB, 128, H // 2, 2, SQ)
            .transpose(2, 0, 1, 3, 4)
            .reshape(H // 2, KB, 128, 2 * SQ)
        ).astype(E4)
        mask_c = src_mask[b, 0, q0:q1, :][:, perm]  # [SQ, S]
        maskT = np.ascontiguousarray(
            mask_c.T.astype(np.float32) * MASKV
        ).reshape(KB, 128, SQ).astype(E4)
        m = dict(common)
        m["srcT"] = srcT
        m["biasT"] = biasT
        m["maskT"] = maskT
        m["tstep"] = timestep[b].reshape(1, 1)
        in_maps.append(m)
    return in_maps


def assemble_output(results):
    out = np.empty((B, S, D), dtype=np.float32)
    for core in range(NC):
        b, j = core // 2, core % 2
        o = np.asarray(results[core]["outT"], dtype=np.float32)  # [DC, 128, SQ]
        out[b, SQ * j : SQ * (j + 1), :] = o.reshape(D, SQ).T
    return out


def run(inputs, trace=False, **kw):
    from concourse import bass_utils

    nc = build_nc()
    in_maps = make_in_maps(inputs)
    res = bass_utils.run_bass_kernel_spmd(
        nc, in_maps, list(range(NC)), trace=trace, **kw
    )
    return assemble_output(res.results), res


def kernel(**inputs):
    out, _ = run(inputs)
    return out
